# revision 23
# baseline (speedup 1.0000x reference)
"""Bahdanau additive attention on 8 TRN2 NeuronCores.

B=8, L=512, D=128. Data-parallel: one batch element per core, no collectives.

Per-core layout: feature dim d (=128) on partitions.
  - WhT/WsT = (Wh_w @ H_b^T), (Ws_w @ S_b^T): (128 d', 512 seq) via PE fp32.
  - tanh inputs built by DVE per-partition adds: SUM[:, i-slot] = WsT + WhT[:, i]
    (fp32 tensor_scalar -> 2x mode), grouped K8 queries per (128, K8*512) tile.
  - ACT tanh on the big tile -> bf16 T.
  - PE projects each (128,512) T slice against V with a sliding-column lhsT
    (V at column 128 of a zero (128,512) buffer; slice [128-r : 256-r] puts V
    in column r) accumulating score rows into a (128 i, 512 j) PSUM tile.
  - Softmax without max-subtraction (scores bounded by sum|V| ~ 5.7):
    scoreS = mask*-1e30 + psum (one DVE scalar_tensor_tensor), ACT exp with
    accum_out giving row sums, DVE reciprocal.
  - PE transposes E -> E^T, final matmul  rSeq = E^T.T @ H  (fp32), scale rows
    by 1/sum, DMA out.
"""

import os
import sys

if "/opt/trn_rl_repo" not in sys.path:
    sys.path.insert(0, "/opt/trn_rl_repo")
os.environ.setdefault("MYCRO_LOCAL_CACHE", "1")

import numpy as np

B, L, D = 8, 512, 128
K8 = 16           # queries per tanh group (ACT tile = (128, K8*512))
NBLK = L // 128   # query blocks per batch
VSPLIT = False    # True: split V into bf16 hi+lo passes (2x proj matmuls)
SUM_BF16 = False   # bf16 tanh inputs + in-place tanh (saves ACT instr overhead)

_nc_cache = {}


def _build_nc(repeat=1):
    import concourse.tile as tile
    from concourse import bacc, mybir
    from concourse.masks import make_identity

    FP32 = mybir.dt.float32
    BF16 = mybir.dt.bfloat16
    U8 = mybir.dt.uint8
    Alu = mybir.AluOpType
    Act = mybir.ActivationFunctionType

    nc = bacc.Bacc()
    Hn_d = nc.declare_dram_parameter("Hn", [L, D], FP32, isOutput=False)
    HT_d = nc.declare_dram_parameter("HT", [D, L], FP32, isOutput=False)
    ST_d = nc.declare_dram_parameter("ST", [D, L], FP32, isOutput=False)
    mk_d = nc.declare_dram_parameter("mask", [L, L], U8, isOutput=False)
    WhwT_d = nc.declare_dram_parameter("WhwT", [D, D], FP32, isOutput=False)
    WswT_d = nc.declare_dram_parameter("WswT", [D, D], FP32, isOutput=False)
    V_d = nc.declare_dram_parameter("V", [D, 1], FP32, isOutput=False)
    out_d = nc.declare_dram_parameter("out", [L, D], FP32, isOutput=True)

    with tile.TileContext(nc) as tc:
        with (
            tc.tile_pool(name="const", bufs=1) as cpool,
            tc.tile_pool(name="sum", bufs=3) as spool,
            tc.tile_pool(name="tt", bufs=3) as tpool,
            tc.tile_pool(name="sm", bufs=2) as sm,
            tc.tile_pool(name="psc", bufs=2, space="PSUM") as pbig,
            tc.tile_pool(name="psm", bufs=2, space="PSUM") as psmall,
        ):
            I128 = cpool.tile([128, 128], FP32)
            make_identity(nc, I128[:])

            def emit_once():
                # ---- per-batch prep (latency-critical DMAs first) ----
                HT_s = cpool.tile([128, 512], FP32, tag="HT")
                nc.sync.dma_start(HT_s[:], HT_d[:])
                ST_s = cpool.tile([128, 512], FP32, tag="ST")
                nc.sync.dma_start(ST_s[:], ST_d[:])
                WhwT = cpool.tile([128, 128], FP32, tag="Whw")
                nc.sync.dma_start(WhwT[:], WhwT_d[:])
                WswT = cpool.tile([128, 128], FP32, tag="Wsw")
                nc.sync.dma_start(WswT[:], WswT_d[:])
                Vcol = cpool.tile([128, 1], FP32, tag="Vc")
                nc.sync.dma_start(Vcol[:], V_d[:])
                Hn = cpool.tile([128, NBLK, 128], FP32, tag="Hn")
                nc.sync.dma_start(
                    Hn[:], Hn_d[:].rearrange("(a p) d -> p a d", p=128)
                )
                mask_all = cpool.tile([128, NBLK, 512], U8, tag="mask")
                nc.sync.dma_start(
                    mask_all[:], mk_d[:].rearrange("(a p) j -> p a j", p=128)
                )

                # PE ramp warmup (cheap transpose before the fp32 prep MMs)
                pwu = psmall.tile([128, 128], FP32, tag="pt")
                nc.tensor.transpose(pwu[:], I128[:], I128[:])

                # WhT[e, i] = sum_d Wh_w[e, d] H[i, d]  (same for S).
                # WsT first (the adds read it whole), then the first 16 WhT
                # columns via a mini-matmul so the first add group can start
                # before the full WhT matmul finishes.
                WhT = cpool.tile([128, 512], FP32, tag="WhT")
                WsT = cpool.tile([128, 512], FP32, tag="WsT")
                ps_wm = psmall.tile([128, 128], FP32, tag="pr")
                nc.tensor.matmul(ps_wm[:, :16], WhwT[:], HT_s[:, :16])
                nc.vector.tensor_copy(WhT[:, :16], ps_wm[:, :16])
                ps_w2 = pbig.tile([128, 512], FP32, tag="sc")
                nc.tensor.matmul(ps_w2[:], WswT[:], ST_s[:])
                nc.vector.tensor_copy(WsT[:], ps_w2[:])
                ps_w = pbig.tile([128, 512], FP32, tag="sc")
                nc.tensor.matmul(ps_w[:, : 512 - 16], WhwT[:], HT_s[:, 16:])
                nc.vector.tensor_copy(WhT[:, 16:], ps_w[:, : 512 - 16])

                # sliding V-column buffer: zeros, V(bf16) at col 128 (hi),
                # V - bf16(V) at col 384 (lo, VSPLIT only).
                Vbuf = cpool.tile([128, 512], BF16, tag="Vbuf")
                nc.vector.memset(Vbuf[:], 0.0)
                nc.vector.tensor_copy(Vbuf[:, 128:129], Vcol[:])
                if VSPLIT:
                    Vhi32 = cpool.tile([128, 1], FP32, tag="Vhi")
                    nc.vector.tensor_copy(Vhi32[:], Vbuf[:, 128:129])
                    Vlo32 = cpool.tile([128, 1], FP32, tag="Vlo")
                    nc.vector.tensor_tensor(
                        out=Vlo32[:], in0=Vcol[:], in1=Vhi32[:], op=Alu.subtract
                    )
                    nc.vector.tensor_copy(Vbuf[:, 384:385], Vlo32[:])

                # ---- main loop over query blocks ----
                def group_plan(ib):
                    # Ramp up at kernel start (ACT can begin after 2 adds) and
                    # taper at the very end (shorter proj->softmax tail).
                    gs = [2, 2, 4, 8, 16]
                    rest = 128 - sum(gs)
                    gs += [K8] * (rest // K8) + ([rest % K8] if rest % K8 else [])
                    if ib == 0:
                        return gs
                    if ib == NBLK - 1:
                        return list(reversed(gs))
                    return [K8] * (128 // K8)

                def emit_groups(ib, mid_cb=None):
                    ps = pbig.tile([128, 512], FP32, tag="sc")
                    r = 0
                    for gi, gsz in enumerate(group_plan(ib)):
                        if gi == 2 and mid_cb is not None:
                            mid_cb()
                        SUM = spool.tile(
                            [128, K8 * 512], BF16 if SUM_BF16 else FP32, tag="sum"
                        )
                        for i2 in range(gsz):
                            i = ib * 128 + r + i2
                            nc.vector.tensor_scalar_add(
                                SUM[:, i2 * 512 : (i2 + 1) * 512],
                                WsT[:],
                                WhT[:, i : i + 1],
                            )
                        if SUM_BF16:
                            T = SUM
                            nc.scalar.activation(
                                SUM[:, : gsz * 512], SUM[:, : gsz * 512], Act.Tanh
                            )
                        else:
                            T = tpool.tile([128, K8 * 512], BF16, tag="tt")
                            nc.scalar.activation(
                                T[:, : gsz * 512], SUM[:, : gsz * 512], Act.Tanh
                            )
                        for i2 in range(gsz):
                            rr = r + i2
                            last = rr == 127
                            nc.tensor.matmul(
                                ps[:],
                                Vbuf[:, 128 - rr : 256 - rr],
                                T[:, i2 * 512 : (i2 + 1) * 512],
                                start=(rr == 0),
                                stop=(last and not VSPLIT),
                            )
                            if VSPLIT:
                                nc.tensor.matmul(
                                    ps[:],
                                    Vbuf[:, 384 - rr : 512 - rr],
                                    T[:, i2 * 512 : (i2 + 1) * 512],
                                    start=False,
                                    stop=last,
                                )
                        r += gsz
                    return ps

                def emit_softmax(ib, ps):
                    # softmax over j; masked -> -1e30 -> exp underflows to 0
                    scoreS = sm.tile([128, 512], FP32, tag="scoreS")
                    nc.vector.scalar_tensor_tensor(
                        scoreS[:], mask_all[:, ib, :], -1.0e30, ps[:],
                        Alu.mult, Alu.add,
                    )
                    E = sm.tile([128, 512], FP32, tag="E")
                    sums = sm.tile([128, 1], FP32, tag="sums")
                    nc.scalar.activation(
                        E[:], scoreS[:], Act.Exp, accum_out=sums[:]
                    )
                    rec = sm.tile([128, 1], FP32, tag="rec")
                    nc.vector.reciprocal(rec[:], sums[:])

                    ET = sm.tile([128, 512], FP32, tag="ET")
                    for jb in range(4):
                        pt = psmall.tile([128, 128], FP32, tag="pt")
                        nc.tensor.transpose(
                            pt[:], E[:, jb * 128 : (jb + 1) * 128], I128[:]
                        )
                        nc.vector.tensor_copy(
                            ET[:, jb * 128 : (jb + 1) * 128], pt[:]
                        )

                    pr = psmall.tile([128, 128], FP32, tag="pr")
                    for jb in range(4):
                        nc.tensor.matmul(
                            pr[:],
                            ET[:, jb * 128 : (jb + 1) * 128],
                            Hn[:, jb, :],
                            start=(jb == 0),
                            stop=(jb == 3),
                        )
                    outT = sm.tile([128, 128], FP32, tag="outT")
                    nc.vector.tensor_scalar_mul(outT[:], pr[:], rec[:])
                    nc.sync.dma_start(
                        out_d[ib * 128 : (ib + 1) * 128, :], outT[:]
                    )

                # defer each block's softmax into the NEXT block's group
                # stream (after 2 groups), so ACT's exp never stalls the
                # tanh stream and DVE's mask op isn't queued behind a full
                # block of adds
                pending = []

                def flush_pending():
                    while pending:
                        emit_softmax(*pending.pop(0))

                for ib in range(NBLK):
                    ps = emit_groups(ib, mid_cb=flush_pending)
                    pending.append((ib, ps))
                flush_pending()

            for _rep in range(repeat):
                emit_once()

    nc.compile()
    return nc


def _get_nc(repeat=1):
    if repeat not in _nc_cache:
        _nc_cache[repeat] = _build_nc(repeat)
    return _nc_cache[repeat]


def _run(H, S, mask, Wh_w, Ws_w, V_w, trace=False):
    from concourse.bass_utils import run_bass_kernel_spmd

    nc = _get_nc()
    H = np.asarray(H, np.float32)
    S = np.asarray(S, np.float32)
    mask_u8 = np.ascontiguousarray(mask).astype(np.uint8)
    WhwT = np.ascontiguousarray(np.asarray(Wh_w, np.float32).T)
    WswT = np.ascontiguousarray(np.asarray(Ws_w, np.float32).T)
    Vc = np.ascontiguousarray(np.asarray(V_w, np.float32).reshape(D, 1))
    in_maps = []
    for b in range(B):
        in_maps.append(
            {
                "Hn": np.ascontiguousarray(H[b]),
                "HT": np.ascontiguousarray(H[b].T),
                "ST": np.ascontiguousarray(S[b].T),
                "mask": mask_u8[b],
                "WhwT": WhwT,
                "WswT": WswT,
                "V": Vc,
            }
        )
    res = run_bass_kernel_spmd(nc, in_maps, list(range(B)), trace=trace)
    out = np.stack([res.results[i]["out"] for i in range(B)], axis=0)
    return out.astype(np.float32), res


def kernel(H, S, mask, Wh_w, Ws_w, V_w):
    out, _ = _run(H, S, mask, Wh_w, Ws_w, V_w, trace=False)
    return out
